# revision 26
# baseline (speedup 1.0000x reference)
"""Trainium2 Bass kernel for nn_Attention_59047210385633.

2-D RoPE multi-head attention (B=2, N=2305, D=768, H=12, E=64), sharded
over 8 NeuronCores: each core gets one batch and 3 heads (data parallel
on B x tensor parallel on H). Host sums the 4 partial wo-projections per
batch.

Per-core device kernel (all matmuls fp32r = full-rate single-pass fp32):
  1. qkvT [576, N] = wqkvT.T @ xT  (Q,K per-head rows permuted evens|odds)
  2. RoPE on qT/kT via cos/sin tables (CLS column = identity rotation)
  3. per head: sT = kT.T @ qT / 8 -> exp (no max-sub; scores are small)
     -> PV with ones-augmented V giving denominators in row 64
  4. normalize, project through woT, DMA partial out from PSUM.

Self-contained: hardcodes all shapes; only needs numpy + concourse.
"""

import numpy as np

import bass_rust
import concourse.bass as bass
import concourse.mybir as mybir
import concourse.tile as tile
from concourse.bass_utils import run_bass_kernel_spmd

FP32 = mybir.dt.float32
FP32R = mybir.dt.float32r
AF = mybir.ActivationFunctionType
OP = mybir.AluOpType

B, N, D, H, E = 2, 2305, 768, 12, 64
NP = 2306  # padded token count (fp32r matmuls need even free sizes)
KMAX = 16
BASE = 10000.0
N1 = N2 = 48
HPC = 3  # heads per core

# token panels (q-tiles) and nk chunks (over padded token range)
PANELS = [(0, 512), (512, 512), (1024, 512), (1536, 512), (2048, 258)]
NKCH = [(i * 128, 128) for i in range(18)] + [(2304, 2)]
# qkv output row chunks of wqkvT's 576 columns
MCH = [(0, 128), (128, 128), (256, 128), (384, 128), (512, 64)]
VE = 66   # per-head block in v_sb: 64 v cols + 1 ones col + 1 pad col
VS = 3 * VE  # per-nk-chunk stride in v_sb


def split_excess_waits(nc):
    """walrus CoreV3 codegen allows only one sync wait per engine
    instruction; move excess waits onto NoOps inserted just before."""
    engines = {
        mybir.EngineType.PE,
        mybir.EngineType.DVE,
        mybir.EngineType.Activation,
        mybir.EngineType.Pool,
        mybir.EngineType.SP,
    }
    for f in nc.m.functions:
        for b in f.blocks:
            newl = []
            changed = False
            for ins in b.instructions:
                si = ins.sync_info
                if (
                    si is not None
                    and si.on_wait is not None
                    and len(si.on_wait) > 1
                    and ins.engine in engines
                ):
                    waits = list(si.on_wait)
                    for j, w in enumerate(waits[:-1]):
                        nop = bass_rust.InstNoOp(
                            name=f"{ins.name}-wf{j}", ins=[], outs=[]
                        )
                        nop.engine = ins.engine
                        nop.sync_info = mybir.SyncInfo(on_wait=[w], on_update=[])
                        newl.append(nop)
                    ins.sync_info = mybir.SyncInfo(
                        on_wait=[waits[-1]], on_update=list(si.on_update or [])
                    )
                    changed = True
                newl.append(ins)
            if changed:
                b.instructions = newl


def _emit(nc, tc, ctx, phases=3):
    from concourse.masks import make_identity

    xT = nc.dram_tensor("xT", [D, NP], FP32, kind="ExternalInput").ap()
    wqkvT = nc.dram_tensor("wqkvT", [D, 576], FP32, kind="ExternalInput").ap()
    woT = nc.dram_tensor("woT", [192, D], FP32, kind="ExternalInput").ap()
    cc = nc.dram_tensor("cc", [128, NP], FP32, kind="ExternalInput").ap()
    ss = nc.dram_tensor("ss", [128, NP], FP32, kind="ExternalInput").ap()
    onesd = nc.dram_tensor("onesd", [65, 64], FP32, kind="ExternalInput").ap()
    onescol = nc.dram_tensor("onescol", [128, 57], FP32, kind="ExternalInput").ap()
    out = nc.dram_tensor("out", [NP, D], FP32, kind="ExternalOutput").ap()

    const = ctx.enter_context(tc.tile_pool(name="const", bufs=1))
    xpool = ctx.enter_context(tc.tile_pool(name="xp", bufs=2))
    tcsp = ctx.enter_context(tc.tile_pool(name="tcs", bufs=2))
    ptp = ctx.enter_context(tc.tile_pool(name="pt", bufs=3))
    unp = ctx.enter_context(tc.tile_pool(name="un", bufs=4))
    smallp = ctx.enter_context(tc.tile_pool(name="small", bufs=2))
    abp = ctx.enter_context(tc.tile_pool(name="ab", bufs=2))
    outp_pool = ctx.enter_context(tc.tile_pool(name="osb", bufs=3))

    ps_sg = ctx.enter_context(tc.tile_pool(name="ps_sg", bufs=2, space="PSUM"))
    ps_po = ctx.enter_context(tc.tile_pool(name="ps_po", bufs=2, space="PSUM"))
    ps_rp = ctx.enter_context(tc.tile_pool(name="ps_rp", bufs=2, space="PSUM"))

    # ---- constants -------------------------------------------------------
    wq_sb = const.tile([128, 6, 576], FP32R)
    wqr = wqkvT.rearrange("(c p) m -> p c m", p=128).bitcast(FP32R)
    nc.sync.dma_start(out=wq_sb[:, 0:1, :], in_=wqr[:, 0:1, :])
    cc_sb = const.tile([128, NP], FP32)
    ss_sb = const.tile([128, NP], FP32)
    wo01 = const.tile([128, D], FP32R)
    wo2 = const.tile([64, D], FP32R)
    ident = const.tile([128, 128], FP32)
    make_identity(nc, ident[:])
    # replicate-matmul operands must sit at base partition 0/32/64, so the
    # per-head denominator rows live at partitions 0, 32, 64.
    # (memset cannot write fp32r -> DMA the ones from DRAM)
    ones3 = const.tile([65, 64], FP32R)

    # persistent q/k (rope-rotated, fp32r) and v staging
    qA = const.tile([128, NP], FP32R)  # q_h0 | q_h1
    kA = const.tile([128, NP], FP32R)  # k_h0 | k_h1
    qB = const.tile([64, NP], FP32R)   # q_h2
    kB = const.tile([64, NP], FP32R)   # k_h2
    vt01 = const.tile([128, NP], FP32)  # vT h0|h1 staging
    vt2 = const.tile([64, NP], FP32)    # vT h2 staging
    v_sb = const.tile([128, 19 * VS], FP32R)  # [nk-part, 19 x (3 x 66)]
    v_sb4 = v_sb.rearrange("p (c h e) -> p c h e", c=19, h=HPC)
    # ones columns at offset 64 within each 66-block; the pad key (row 1 of
    # chunk 18) gets 0 so it contributes nothing to the denominators
    nc.sync.dma_start(
        out=v_sb4[:, :, :, 64:65],
        in_=onescol.rearrange("p (c h) -> p c h", c=19).bitcast(FP32R),
    )

    # rope B-op target map: (m0, g) -> (tile, row base)
    rope_tgt = {
        (0, 0): (qA, 0), (0, 1): (qA, 64),
        (128, 0): (qB, 0), (128, 1): (kA, 0),
        (256, 0): (kA, 64), (256, 1): (kB, 0),
    }

    # ---- phase 1: QKV projection + rope + v staging ----------------------
    # SP-ring DMA order matters: weights, first x panel, rope tables, then
    # the remaining panels; wo/ones ride later (needed only in phase 2).
    xTr = xT.rearrange("(c p) n -> p c n", p=128).bitcast(FP32R)
    MCH_VFIRST = [(384, 128), (512, 64), (0, 128), (128, 128), (256, 128)]
    for pi, (off, w) in enumerate(PANELS):
        xp = xpool.tile([128, 6, 512], FP32R, tag="xp")
        if pi == 0:
            nc.sync.dma_start(out=xp[:, 0:1, :w], in_=xTr[:, 0:1, off:off + w])
            nc.sync.dma_start(out=wq_sb[:, 1:6, :], in_=wqr[:, 1:6, :])
            nc.sync.dma_start(out=xp[:, 1:6, :w], in_=xTr[:, 1:6, off:off + w])
            nc.sync.dma_start(out=cc_sb, in_=cc)
            nc.sync.dma_start(out=ss_sb, in_=ss)
        else:
            nc.sync.dma_start(out=xp[:, :, :w], in_=xTr[:, :, off:off + w])
        if False:
            pass
        elif pi == 1:
            nc.sync.dma_start(out=wo01, in_=woT[0:128, :].bitcast(FP32R))
            nc.sync.dma_start(out=wo2, in_=woT[128:192, :].bitcast(FP32R))
            nc.sync.dma_start(out=ones3, in_=onesd.bitcast(FP32R))
        for m0, mw in MCH_VFIRST:
            qp = ps_rp.tile([128, 512], FP32, tag="rp")
            for kc in range(6):
                nc.tensor.matmul(
                    qp[:mw, :w],
                    lhsT=wq_sb[:, kc, m0:m0 + mw],
                    rhs=xp[:, kc, :w],
                    start=(kc == 0),
                    stop=(kc == 5),
                )
            if m0 == 0:
                # q_h0|q_h1 rope routed via GPSIMD: tcs = q*cos, tss = q*sin
                # (both SBUF, DVE), gpsimd swap-copies tss 32-row blocks to
                # tssw with the sign folded into a subtract/add pair, then
                # one add per 64-block. Keeps DVE short so k-rope (which
                # gates every attention chain) finishes earlier.
                tcs = tcsp.tile([128, 512], FP32, tag="tcs")
                nc.vector.tensor_tensor(
                    out=tcs[:, :w], in0=qp[:, :w], in1=cc_sb[:, off:off + w],
                    op=OP.mult,
                )
                tss = tcsp.tile([128, 512], FP32, tag="tss")
                nc.vector.tensor_tensor(
                    out=tss[:, :w], in0=qp[:, :w], in1=ss_sb[:, off:off + w],
                    op=OP.mult,
                )
                # tssw[r] = -tss[i] = -qi*sin ; tssw[i] = tss[r] = qr*sin
                tssw = tcsp.tile([128, 512], FP32, tag="tssw")
                for g in range(2):
                    r = slice(g * 64, g * 64 + 32)
                    i = slice(g * 64 + 32, g * 64 + 64)
                    nc.gpsimd.tensor_scalar_mul(tssw[r, :w], tss[i, :w], -1.0)
                    nc.gpsimd.tensor_copy(tssw[i, :w], tss[r, :w])
                nc.gpsimd.tensor_tensor(
                    out=qA[0:128, off:off + w],
                    in0=tcs[:, :w], in1=tssw[:, :w], op=OP.add,
                )
            elif m0 < 384:
                # k-rope (and q_h2) on DVE: tcs = q*cos (SBUF), tsp = q*sin
                # (PSUM), then rot_r = tcs[r]-tsp[i]; rot_i = tcs[i]+tsp[r]
                tcs = tcsp.tile([128, 512], FP32, tag="tcs")
                nc.vector.tensor_tensor(
                    out=tcs[:, :w], in0=qp[:, :w], in1=cc_sb[:, off:off + w],
                    op=OP.mult,
                )
                tsp = ps_sg.tile([128, 1024], FP32, tag="sg")
                nc.vector.tensor_tensor(
                    out=tsp[:, :w], in0=qp[:, :w], in1=ss_sb[:, off:off + w],
                    op=OP.mult,
                )
                for g in range(2):
                    tgt, base = rope_tgt[(m0, g)]
                    r = slice(g * 64, g * 64 + 32)
                    i = slice(g * 64 + 32, g * 64 + 64)
                    nc.vector.tensor_tensor(
                        out=tgt[base:base + 32, off:off + w],
                        in0=tcs[r, :w], in1=tsp[i, :w], op=OP.subtract,
                    )
                    nc.vector.tensor_tensor(
                        out=tgt[base + 32:base + 64, off:off + w],
                        in0=tcs[i, :w], in1=tsp[r, :w], op=OP.add,
                    )
            elif m0 == 384:
                nc.vector.tensor_copy(vt01[:, off:off + w], qp[:, :w])
            else:
                nc.vector.tensor_copy(vt2[:, off:off + w], qp[:64, :w])
        # transpose this panel's chunks of vT -> v_sb right away so the PE
        # reaches attention as soon as the last panel's rope is done.
        # NB: each matmul accumulation group must own its PSUM bank - two
        # transposes into different columns of one bank crash the device.
        for ci, (c0, cw) in enumerate(NKCH):
            if not (off <= c0 < off + w):
                continue
            vt_ps = ps_rp.tile([128, 512], FP32, tag="rp", name=f"vtp{ci}")
            nc.tensor.transpose(
                vt_ps[:cw, 0:128], vt01[:, c0:c0 + cw], ident[:, :]
            )
            vt_ps2 = ps_sg.tile([128, 1024], FP32, tag="sg", name=f"vtp2{ci}")
            nc.tensor.transpose(
                vt_ps2[:cw, 0:64], vt2[0:64, c0:c0 + cw], ident[0:64, 0:64]
            )
            nc.vector.tensor_copy(
                v_sb4[0:cw, ci, 0:2, 0:64],
                vt_ps[:cw, 0:128].rearrange("p (h e) -> p h e", h=2),
            )
            nc.vector.tensor_copy(
                v_sb4[0:cw, ci, 2, 0:64], vt_ps2[:cw, 0:64]
            )

    if phases == 1:
        dqa = nc.dram_tensor("dqa", [128, NP], FP32, kind="ExternalOutput").ap()
        dka = nc.dram_tensor("dka", [128, NP], FP32, kind="ExternalOutput").ap()
        dqb = nc.dram_tensor("dqb", [64, NP], FP32, kind="ExternalOutput").ap()
        dkb = nc.dram_tensor("dkb", [64, NP], FP32, kind="ExternalOutput").ap()
        dvs = nc.dram_tensor("dvs", [128, 19 * VS], FP32, kind="ExternalOutput").ap()
        nc.sync.dma_start(out=dqa, in_=qA.bitcast(FP32))
        nc.sync.dma_start(out=dka, in_=kA.bitcast(FP32))
        nc.sync.dma_start(out=dqb, in_=qB.bitcast(FP32))
        nc.sync.dma_start(out=dkb, in_=kB.bitcast(FP32))
        nc.sync.dma_start(out=dvs, in_=v_sb.bitcast(FP32))
        return

    # ---- phase 2: attention per (q-tile, head) ---------------------------
    qk_of_head = [(qA, kA, 0), (qA, kA, 64), (qB, kB, 0)]
    if phases == 2:
        dun = nc.dram_tensor("dun", [3 * 64, 512], FP32, kind="ExternalOutput").ap()
        dden = nc.dram_tensor("dden", [65, 512], FP32, kind="ExternalOutput").ap()
        drec = nc.dram_tensor("drec", [65, 512], FP32, kind="ExternalOutput").ap()
        dab = nc.dram_tensor("dab", [192, 512], FP32, kind="ExternalOutput").ap()
        dpt = nc.dram_tensor("dpt", [128, 1024], FP32, kind="ExternalOutput").ap()
    for off, w in PANELS:
        if phases == 2 and off > 0:
            break
        den = smallp.tile([65, 512], FP32, tag="den")
        un_tiles = {}
        outps = {}

        def attn_group(h, ci, npair):
            qt, kt, hb = qk_of_head[h]
            sg = ps_sg.tile([128, 1024], FP32, tag="sg", name=f"sg{h}_{ci}")
            for half in range(npair):
                c0, cw = NKCH[ci + half]
                nc.tensor.matmul(
                    sg[:cw, half * 512: half * 512 + w],
                    lhsT=kt[hb:hb + 64, c0:c0 + cw],
                    rhs=qt[hb:hb + 64, off:off + w],
                    start=True,
                    stop=True,
                )
            pt = ptp.tile([128, 1024], FP32R, tag="pt", name=f"pt{h}_{ci}")
            if npair == 2:
                sg_v = sg.rearrange("p (g c) -> p g c", g=2)[:, :, :w]
                pt_v = pt.rearrange("p (g c) -> p g c", g=2)[:, :, :w]
                nc.scalar.activation(pt_v, sg_v, AF.Exp, scale=0.125)
            else:
                c0, cw = NKCH[ci]
                nc.scalar.activation(
                    pt[:cw, :w], sg[:cw, :w], AF.Exp, scale=0.125
                )
            for half in range(npair):
                c0, cw = NKCH[ci + half]
                nc.tensor.matmul(
                    outps[h][0:66, :w],
                    lhsT=v_sb[:cw, (ci + half) * VS + h * VE:
                              (ci + half) * VS + h * VE + VE],
                    rhs=pt[:cw, half * 512: half * 512 + w],
                    start=(ci + half == 0),
                    stop=(ci + half == len(NKCH) - 1),
                )

        def finish_head(h):
            nc.vector.tensor_copy(
                den[32 * h:32 * h + 1, :w], outps[h][64:65, :w]
            )
            un = unp.tile([64, 512], FP32, tag="un", name=f"un{h}")
            nc.vector.tensor_copy(un[:, :w], outps[h][0:64, :w])
            un_tiles[h] = un

        groups = []
        ci = 0
        while ci < len(NKCH):
            npair = 2 if ci + 1 < len(NKCH) else 1
            groups.append((ci, npair))
            ci += npair
        # heads 0,1 interleaved (PE stays busy during the other head's exp),
        # then head 2 (PSUM budget allows only 2 open PV accumulators)
        outps[0] = ps_po.tile([66, 512], FP32, tag="po", name="outp0")
        outps[1] = ps_po.tile([66, 512], FP32, tag="po", name="outp1")
        for ci, npair in groups:
            attn_group(0, ci, npair)
            attn_group(1, ci, npair)
        finish_head(0)
        finish_head(1)
        outps[2] = ps_po.tile([66, 512], FP32, tag="po", name="outp2")
        for ci, npair in groups:
            attn_group(2, ci, npair)
        finish_head(2)
        if phases == 2:
            for h in range(HPC):
                nc.sync.dma_start(
                    out=dun[64 * h:64 * h + 64, :], in_=un_tiles[h][:, :]
                )
            nc.sync.dma_start(out=dden, in_=den[:, :])
            return
        rec = smallp.tile([65, 512], FP32R, tag="rec")
        with nc.allow_low_precision(reason="fp32r is 4-byte; rounding only"):
            nc.vector.reciprocal(rec[:, :w], den[:, :w])
        if phases == 2:
            nc.sync.dma_start(out=dden, in_=den[:, :])
            nc.sync.dma_start(out=drec, in_=rec.bitcast(FP32))
        ab01 = abp.tile([128, 512], FP32R, tag="ab01")
        ab2 = abp.tile([64, 512], FP32R, tag="ab2")
        ab_of_head = [(ab01, 0), (ab01, 64), (ab2, 0)]
        for h in range(HPC):
            rep = ps_rp.tile([128, 512], FP32, tag="rp")
            nc.tensor.matmul(
                rep[0:64, :w],
                lhsT=ones3[32 * h:32 * h + 1, 0:64],
                rhs=rec[32 * h:32 * h + 1, :w],
                start=True,
                stop=True,
            )
            abt, ab_base = ab_of_head[h]
            nc.vector.tensor_tensor(
                out=abt[ab_base:ab_base + 64, :w],
                in0=un_tiles[h][:, :w],
                in1=rep[0:64, :w],
                op=OP.mult,
            )
        if phases == 2:
            nc.sync.dma_start(out=dab[0:128, :], in_=ab01.bitcast(FP32))
            nc.sync.dma_start(out=dab[128:192, :], in_=ab2.bitcast(FP32))
            return
        # output projection for this q-tile; DMA straight from PSUM
        t0 = 0
        while t0 < w:
            tw = min(128, w - t0)
            for half in range(2):
                op_ps = ps_rp.tile([128, 512], FP32, tag="rp")
                nc.tensor.matmul(
                    op_ps[:tw, :384],
                    lhsT=ab01[:, t0:t0 + tw],
                    rhs=wo01[:, half * 384:half * 384 + 384],
                    start=True,
                    stop=False,
                )
                nc.tensor.matmul(
                    op_ps[:tw, :384],
                    lhsT=ab2[0:64, t0:t0 + tw],
                    rhs=wo2[:, half * 384:half * 384 + 384],
                    start=False,
                    stop=True,
                )
                osb = outp_pool.tile([128, 384], FP32, tag="osb")
                nc.vector.tensor_copy(osb[:tw, :], op_ps[:tw, :384])
                nc.sync.dma_start(
                    out=out[off + t0:off + t0 + tw,
                            half * 384:half * 384 + 384],
                    in_=osb[:tw, :],
                )
            t0 += tw


_NC_CACHE = {}


def build_nc(trace_sim=False, phases=3):
    key = (bool(trace_sim), phases)
    if key in _NC_CACHE:
        return _NC_CACHE[key]
    from contextlib import ExitStack

    nc = bass.Bass("TRN2", target_bir_lowering=False, debug=False, num_devices=8)
    with tile.TileContext(nc, trace_sim=trace_sim) as tc:
        with ExitStack() as ctx:
            _emit(nc, tc, ctx, phases=phases)
    split_excess_waits(nc)
    _NC_CACHE[key] = nc
    return nc


def host_prep(x, pos0, pos1, wq, wk, wv, wo, core):
    """Per-core DRAM inputs. core -> batch b=core//4, heads 3*(core%4)+[0..2]."""
    b = core // 4
    h0 = 3 * (core % 4)
    hs = [h0, h0 + 1, h0 + 2]

    xT = np.zeros((D, NP), np.float32)
    xT[:, :N] = x[b].T

    def perm_rows(w_h):  # evens then odds of the head dim
        return np.concatenate([w_h[0::2], w_h[1::2]], axis=0)

    wq_rows = np.concatenate([perm_rows(wq[h * E:(h + 1) * E]) for h in hs], 0)
    wk_rows = np.concatenate([perm_rows(wk[h * E:(h + 1) * E]) for h in hs], 0)
    wv_rows = np.concatenate([wv[h * E:(h + 1) * E] for h in hs], 0)
    wqkvT = np.ascontiguousarray(np.concatenate([wq_rows, wk_rows, wv_rows], 0).T)

    wo_cols = np.concatenate([wo[:, h * E:(h + 1) * E] for h in hs], 1)
    woT = np.ascontiguousarray(wo_cols.T)

    theta = 1.0 / (BASE ** (np.arange(KMAX, dtype=np.float32) / KMAX))
    i1, i2 = np.meshgrid(np.arange(N1), np.arange(N2), indexing="ij")
    ang0 = pos0[b][i1.ravel()][:, None] * theta[None, :]
    ang1 = pos1[b][i2.ravel()][:, None] * theta[None, :]
    ang = np.concatenate([ang0, ang1], 1).astype(np.float32)  # [N-1, 32]
    cos = np.ones((32, NP), np.float32)   # col 0 (CLS) and pad col: identity
    sin = np.zeros((32, NP), np.float32)
    cos[:, 1:N] = np.cos(ang).T
    sin[:, 1:N] = np.sin(ang).T
    cc = np.ascontiguousarray(np.tile(cos, (4, 1)))  # [128, NP]
    ss = np.ascontiguousarray(np.tile(sin, (4, 1)))
    onesd = np.ones((65, 64), np.float32)
    onescol = np.ones((128, 19, HPC), np.float32)
    onescol[1:, 18, :] = 0.0  # pad key contributes nothing
    return {"xT": xT, "wqkvT": wqkvT, "woT": woT, "cc": cc, "ss": ss,
            "onesd": onesd, "onescol": np.ascontiguousarray(onescol.reshape(128, 57))}


def kernel(x, pos0, pos1, wq, wk, wv, wo):
    x = np.asarray(x, np.float32)
    pos0 = np.asarray(pos0, np.float32)
    pos1 = np.asarray(pos1, np.float32)
    wq = np.asarray(wq, np.float32)
    wk = np.asarray(wk, np.float32)
    wv = np.asarray(wv, np.float32)
    wo = np.asarray(wo, np.float32)

    in_maps = [
        host_prep(x, pos0, pos1, wq, wk, wv, wo, core) for core in range(8)
    ]
    nc = build_nc()
    res = run_bass_kernel_spmd(nc, in_maps, core_ids=list(range(8)))
    out = np.zeros((B, N, D), np.float32)
    for core in range(8):
        out[core // 4] += res.results[core]["out"][:N]
    return out


# revision 27
# speedup vs baseline: 1.0820x; 1.0820x over previous
"""Trainium2 Bass kernel for nn_Attention_59047210385633.

2-D RoPE multi-head attention (B=2, N=2305, D=768, H=12, E=64), sharded
over 8 NeuronCores: each core gets one batch and 3 heads (data parallel
on B x tensor parallel on H). Host sums the 4 partial wo-projections per
batch.

Per-core device kernel (all matmuls fp32r = full-rate single-pass fp32):
  1. qkvT [576, N] = wqkvT.T @ xT  (Q,K per-head rows permuted evens|odds)
  2. RoPE on qT/kT via cos/sin tables (CLS column = identity rotation)
  3. per head: sT = kT.T @ qT / 8 -> exp (no max-sub; scores are small)
     -> PV with ones-augmented V giving denominators in row 64
  4. normalize, project through woT, DMA partial out from PSUM.

Self-contained: hardcodes all shapes; only needs numpy + concourse.
"""

import numpy as np

import bass_rust
import concourse.bass as bass
import concourse.mybir as mybir
import concourse.tile as tile
from concourse.bass_utils import run_bass_kernel_spmd

FP32 = mybir.dt.float32
FP32R = mybir.dt.float32r
AF = mybir.ActivationFunctionType
OP = mybir.AluOpType

B, N, D, H, E = 2, 2305, 768, 12, 64
NP = 2306  # padded token count (fp32r matmuls need even free sizes)
KMAX = 16
BASE = 10000.0
N1 = N2 = 48
HPC = 3  # heads per core

# token panels (q-tiles) and nk chunks (over padded token range)
PANELS = [(0, 512), (512, 512), (1024, 512), (1536, 512), (2048, 258)]
NKCH = [(i * 128, 128) for i in range(18)] + [(2304, 2)]
# qkv output row chunks of wqkvT's 576 columns
MCH = [(0, 128), (128, 128), (256, 128), (384, 128), (512, 64)]
VE = 66   # per-head block in v_sb: 64 v cols + 1 ones col + 1 pad col
VS = 3 * VE  # per-nk-chunk stride in v_sb


def split_excess_waits(nc):
    """walrus CoreV3 codegen allows only one sync wait per engine
    instruction; move excess waits onto NoOps inserted just before."""
    engines = {
        mybir.EngineType.PE,
        mybir.EngineType.DVE,
        mybir.EngineType.Activation,
        mybir.EngineType.Pool,
        mybir.EngineType.SP,
    }
    for f in nc.m.functions:
        for b in f.blocks:
            newl = []
            changed = False
            for ins in b.instructions:
                si = ins.sync_info
                if (
                    si is not None
                    and si.on_wait is not None
                    and len(si.on_wait) > 1
                    and ins.engine in engines
                ):
                    waits = list(si.on_wait)
                    for j, w in enumerate(waits[:-1]):
                        nop = bass_rust.InstNoOp(
                            name=f"{ins.name}-wf{j}", ins=[], outs=[]
                        )
                        nop.engine = ins.engine
                        nop.sync_info = mybir.SyncInfo(on_wait=[w], on_update=[])
                        newl.append(nop)
                    ins.sync_info = mybir.SyncInfo(
                        on_wait=[waits[-1]], on_update=list(si.on_update or [])
                    )
                    changed = True
                newl.append(ins)
            if changed:
                b.instructions = newl


def _emit(nc, tc, ctx, phases=3):
    from concourse.masks import make_identity

    xT = nc.dram_tensor("xT", [D, NP], FP32, kind="ExternalInput").ap()
    wqkvT = nc.dram_tensor("wqkvT", [D, 576], FP32, kind="ExternalInput").ap()
    woT = nc.dram_tensor("woT", [192, D], FP32, kind="ExternalInput").ap()
    cc = nc.dram_tensor("cc", [128, NP], FP32, kind="ExternalInput").ap()
    ss = nc.dram_tensor("ss", [128, NP], FP32, kind="ExternalInput").ap()
    onesd = nc.dram_tensor("onesd", [65, 64], FP32, kind="ExternalInput").ap()
    onescol = nc.dram_tensor("onescol", [128, 57], FP32, kind="ExternalInput").ap()
    out = nc.dram_tensor("out", [NP, D], FP32, kind="ExternalOutput").ap()

    const = ctx.enter_context(tc.tile_pool(name="const", bufs=1))
    xpool = ctx.enter_context(tc.tile_pool(name="xp", bufs=2))
    tcsp = ctx.enter_context(tc.tile_pool(name="tcs", bufs=2))
    ptp = ctx.enter_context(tc.tile_pool(name="pt", bufs=3))
    unp = ctx.enter_context(tc.tile_pool(name="un", bufs=4))
    smallp = ctx.enter_context(tc.tile_pool(name="small", bufs=2))
    abp = ctx.enter_context(tc.tile_pool(name="ab", bufs=2))
    outp_pool = ctx.enter_context(tc.tile_pool(name="osb", bufs=3))

    ps_sg = ctx.enter_context(tc.tile_pool(name="ps_sg", bufs=2, space="PSUM"))
    ps_po = ctx.enter_context(tc.tile_pool(name="ps_po", bufs=2, space="PSUM"))
    ps_rp = ctx.enter_context(tc.tile_pool(name="ps_rp", bufs=2, space="PSUM"))

    # ---- constants -------------------------------------------------------
    wq_sb = const.tile([128, 6, 576], FP32R)
    wqr = wqkvT.rearrange("(c p) m -> p c m", p=128).bitcast(FP32R)
    nc.sync.dma_start(out=wq_sb[:, 0:1, :], in_=wqr[:, 0:1, :])
    cc_sb = const.tile([128, NP], FP32)
    ss_sb = const.tile([128, NP], FP32)
    wo01 = const.tile([128, D], FP32R)
    wo2 = const.tile([64, D], FP32R)
    ident = const.tile([128, 128], FP32)
    make_identity(nc, ident[:])
    # replicate-matmul operands must sit at base partition 0/32/64, so the
    # per-head denominator rows live at partitions 0, 32, 64.
    # (memset cannot write fp32r -> DMA the ones from DRAM)
    ones3 = const.tile([65, 64], FP32R)

    # persistent q/k (rope-rotated, fp32r) and v staging
    qA = const.tile([128, NP], FP32R)  # q_h0 | q_h1
    kA = const.tile([128, NP], FP32R)  # k_h0 | k_h1
    qB = const.tile([64, NP], FP32R)   # q_h2
    kB = const.tile([64, NP], FP32R)   # k_h2
    vt01 = const.tile([128, NP], FP32)  # vT h0|h1 staging
    vt2 = const.tile([64, NP], FP32)    # vT h2 staging
    v_sb = const.tile([128, 19 * VS], FP32R)  # [nk-part, 19 x (3 x 66)]
    v_sb4 = v_sb.rearrange("p (c h e) -> p c h e", c=19, h=HPC)
    # ones columns at offset 64 within each 66-block; the pad key (row 1 of
    # chunk 18) gets 0 so it contributes nothing to the denominators
    nc.sync.dma_start(
        out=v_sb4[:, :, :, 64:65],
        in_=onescol.rearrange("p (c h) -> p c h", c=19).bitcast(FP32R),
    )

    # rope B-op target map: (m0, g) -> (tile, row base)
    rope_tgt = {
        (0, 0): (qA, 0), (0, 1): (qA, 64),
        (128, 0): (qB, 0), (128, 1): (kA, 0),
        (256, 0): (kA, 64), (256, 1): (kB, 0),
    }

    # ---- phase 1: QKV projection + rope + v staging ----------------------
    # SP-ring DMA order matters: weights, first x panel, rope tables, then
    # the remaining panels; wo/ones ride later (needed only in phase 2).
    xTr = xT.rearrange("(c p) n -> p c n", p=128).bitcast(FP32R)
    MCH_VFIRST = [(384, 128), (512, 64), (0, 128), (128, 128), (256, 128)]
    for pi, (off, w) in enumerate(PANELS):
        xp = xpool.tile([128, 6, 512], FP32R, tag="xp")
        if pi == 0:
            nc.sync.dma_start(out=xp[:, 0:1, :w], in_=xTr[:, 0:1, off:off + w])
            nc.sync.dma_start(out=wq_sb[:, 1:6, :], in_=wqr[:, 1:6, :])
            nc.sync.dma_start(out=xp[:, 1:6, :w], in_=xTr[:, 1:6, off:off + w])
            nc.sync.dma_start(out=cc_sb, in_=cc)
            nc.sync.dma_start(out=ss_sb, in_=ss)
        else:
            nc.sync.dma_start(out=xp[:, :, :w], in_=xTr[:, :, off:off + w])
        if False:
            pass
        elif pi == 1:
            nc.sync.dma_start(out=wo01, in_=woT[0:128, :].bitcast(FP32R))
            nc.sync.dma_start(out=wo2, in_=woT[128:192, :].bitcast(FP32R))
            nc.sync.dma_start(out=ones3, in_=onesd.bitcast(FP32R))
        for m0, mw in MCH_VFIRST:
            qp = ps_rp.tile([128, 512], FP32, tag="rp")
            for kc in range(6):
                nc.tensor.matmul(
                    qp[:mw, :w],
                    lhsT=wq_sb[:, kc, m0:m0 + mw],
                    rhs=xp[:, kc, :w],
                    start=(kc == 0),
                    stop=(kc == 5),
                )
            if m0 == 0:
                # q_h0|q_h1 rope routed via GPSIMD: tcs = q*cos, tss = q*sin
                # (both SBUF, DVE), gpsimd swap-copies tss 32-row blocks to
                # tssw with the sign folded into a subtract/add pair, then
                # one add per 64-block. Keeps DVE short so k-rope (which
                # gates every attention chain) finishes earlier.
                tcs = tcsp.tile([128, 512], FP32, tag="tcs")
                nc.vector.tensor_tensor(
                    out=tcs[:, :w], in0=qp[:, :w], in1=cc_sb[:, off:off + w],
                    op=OP.mult,
                )
                tss = tcsp.tile([128, 512], FP32, tag="tss")
                nc.vector.tensor_tensor(
                    out=tss[:, :w], in0=qp[:, :w], in1=ss_sb[:, off:off + w],
                    op=OP.mult,
                )
                # tssw[r] = -tss[i] = -qi*sin ; tssw[i] = tss[r] = qr*sin
                tssw = tcsp.tile([128, 512], FP32, tag="tssw")
                for g in range(2):
                    r = slice(g * 64, g * 64 + 32)
                    i = slice(g * 64 + 32, g * 64 + 64)
                    nc.gpsimd.tensor_scalar_mul(tssw[r, :w], tss[i, :w], -1.0)
                    nc.gpsimd.tensor_copy(tssw[i, :w], tss[r, :w])
                nc.gpsimd.tensor_tensor(
                    out=qA[0:128, off:off + w],
                    in0=tcs[:, :w], in1=tssw[:, :w], op=OP.add,
                )
            elif m0 < 384:
                # k-rope (and q_h2) on DVE: tcs = q*cos (SBUF), tsp = q*sin
                # (PSUM), then rot_r = tcs[r]-tsp[i]; rot_i = tcs[i]+tsp[r]
                tcs = tcsp.tile([128, 512], FP32, tag="tcs")
                nc.vector.tensor_tensor(
                    out=tcs[:, :w], in0=qp[:, :w], in1=cc_sb[:, off:off + w],
                    op=OP.mult,
                )
                tsp = ps_sg.tile([128, 1024], FP32, tag="sg")
                nc.vector.tensor_tensor(
                    out=tsp[:, :w], in0=qp[:, :w], in1=ss_sb[:, off:off + w],
                    op=OP.mult,
                )
                for g in range(2):
                    tgt, base = rope_tgt[(m0, g)]
                    r = slice(g * 64, g * 64 + 32)
                    i = slice(g * 64 + 32, g * 64 + 64)
                    nc.vector.tensor_tensor(
                        out=tgt[base:base + 32, off:off + w],
                        in0=tcs[r, :w], in1=tsp[i, :w], op=OP.subtract,
                    )
                    nc.vector.tensor_tensor(
                        out=tgt[base + 32:base + 64, off:off + w],
                        in0=tcs[i, :w], in1=tsp[r, :w], op=OP.add,
                    )
            elif m0 == 384:
                nc.scalar.copy(vt01[:, off:off + w], qp[:, :w])
            else:
                nc.scalar.copy(vt2[:, off:off + w], qp[:64, :w])
        # transpose this panel's chunks of vT -> v_sb right away so the PE
        # reaches attention as soon as the last panel's rope is done.
        # NB: each matmul accumulation group must own its PSUM bank - two
        # transposes into different columns of one bank crash the device.
        for ci, (c0, cw) in enumerate(NKCH):
            if not (off <= c0 < off + w):
                continue
            vt_ps = ps_rp.tile([128, 512], FP32, tag="rp", name=f"vtp{ci}")
            nc.tensor.transpose(
                vt_ps[:cw, 0:128], vt01[:, c0:c0 + cw], ident[:, :]
            )
            vt_ps2 = ps_sg.tile([128, 1024], FP32, tag="sg", name=f"vtp2{ci}")
            nc.tensor.transpose(
                vt_ps2[:cw, 0:64], vt2[0:64, c0:c0 + cw], ident[0:64, 0:64]
            )
            nc.scalar.copy(
                v_sb4[0:cw, ci, 0:2, 0:64],
                vt_ps[:cw, 0:128].rearrange("p (h e) -> p h e", h=2),
            )
            nc.scalar.copy(
                v_sb4[0:cw, ci, 2, 0:64], vt_ps2[:cw, 0:64]
            )

    if phases == 1:
        dqa = nc.dram_tensor("dqa", [128, NP], FP32, kind="ExternalOutput").ap()
        dka = nc.dram_tensor("dka", [128, NP], FP32, kind="ExternalOutput").ap()
        dqb = nc.dram_tensor("dqb", [64, NP], FP32, kind="ExternalOutput").ap()
        dkb = nc.dram_tensor("dkb", [64, NP], FP32, kind="ExternalOutput").ap()
        dvs = nc.dram_tensor("dvs", [128, 19 * VS], FP32, kind="ExternalOutput").ap()
        nc.sync.dma_start(out=dqa, in_=qA.bitcast(FP32))
        nc.sync.dma_start(out=dka, in_=kA.bitcast(FP32))
        nc.sync.dma_start(out=dqb, in_=qB.bitcast(FP32))
        nc.sync.dma_start(out=dkb, in_=kB.bitcast(FP32))
        nc.sync.dma_start(out=dvs, in_=v_sb.bitcast(FP32))
        return

    # ---- phase 2: attention per (q-tile, head) ---------------------------
    qk_of_head = [(qA, kA, 0), (qA, kA, 64), (qB, kB, 0)]
    if phases == 2:
        dun = nc.dram_tensor("dun", [3 * 64, 512], FP32, kind="ExternalOutput").ap()
        dden = nc.dram_tensor("dden", [65, 512], FP32, kind="ExternalOutput").ap()
        drec = nc.dram_tensor("drec", [65, 512], FP32, kind="ExternalOutput").ap()
        dab = nc.dram_tensor("dab", [192, 512], FP32, kind="ExternalOutput").ap()
        dpt = nc.dram_tensor("dpt", [128, 1024], FP32, kind="ExternalOutput").ap()
    for off, w in PANELS:
        if phases == 2 and off > 0:
            break
        den = smallp.tile([65, 512], FP32, tag="den")
        un_tiles = {}
        outps = {}

        def attn_group(h, ci, npair):
            qt, kt, hb = qk_of_head[h]
            sg = ps_sg.tile([128, 1024], FP32, tag="sg", name=f"sg{h}_{ci}")
            for half in range(npair):
                c0, cw = NKCH[ci + half]
                nc.tensor.matmul(
                    sg[:cw, half * 512: half * 512 + w],
                    lhsT=kt[hb:hb + 64, c0:c0 + cw],
                    rhs=qt[hb:hb + 64, off:off + w],
                    start=True,
                    stop=True,
                )
            pt = ptp.tile([128, 1024], FP32R, tag="pt", name=f"pt{h}_{ci}")
            if npair == 2:
                sg_v = sg.rearrange("p (g c) -> p g c", g=2)[:, :, :w]
                pt_v = pt.rearrange("p (g c) -> p g c", g=2)[:, :, :w]
                nc.scalar.activation(pt_v, sg_v, AF.Exp, scale=0.125)
            else:
                c0, cw = NKCH[ci]
                nc.scalar.activation(
                    pt[:cw, :w], sg[:cw, :w], AF.Exp, scale=0.125
                )
            for half in range(npair):
                c0, cw = NKCH[ci + half]
                nc.tensor.matmul(
                    outps[h][0:66, :w],
                    lhsT=v_sb[:cw, (ci + half) * VS + h * VE:
                              (ci + half) * VS + h * VE + VE],
                    rhs=pt[:cw, half * 512: half * 512 + w],
                    start=(ci + half == 0),
                    stop=(ci + half == len(NKCH) - 1),
                )

        def finish_head(h):
            nc.vector.tensor_copy(
                den[32 * h:32 * h + 1, :w], outps[h][64:65, :w]
            )
            un = unp.tile([64, 512], FP32, tag="un", name=f"un{h}")
            nc.vector.tensor_copy(un[:, :w], outps[h][0:64, :w])
            un_tiles[h] = un

        groups = []
        ci = 0
        while ci < len(NKCH):
            npair = 2 if ci + 1 < len(NKCH) else 1
            groups.append((ci, npair))
            ci += npair
        # heads 0,1 interleaved (PE stays busy during the other head's exp),
        # then head 2 (PSUM budget allows only 2 open PV accumulators)
        outps[0] = ps_po.tile([66, 512], FP32, tag="po", name="outp0")
        outps[1] = ps_po.tile([66, 512], FP32, tag="po", name="outp1")
        for ci, npair in groups:
            attn_group(0, ci, npair)
            attn_group(1, ci, npair)
        finish_head(0)
        finish_head(1)
        outps[2] = ps_po.tile([66, 512], FP32, tag="po", name="outp2")
        for ci, npair in groups:
            attn_group(2, ci, npair)
        finish_head(2)
        if phases == 2:
            for h in range(HPC):
                nc.sync.dma_start(
                    out=dun[64 * h:64 * h + 64, :], in_=un_tiles[h][:, :]
                )
            nc.sync.dma_start(out=dden, in_=den[:, :])
            return
        rec = smallp.tile([65, 512], FP32R, tag="rec")
        with nc.allow_low_precision(reason="fp32r is 4-byte; rounding only"):
            nc.vector.reciprocal(rec[:, :w], den[:, :w])
        if phases == 2:
            nc.sync.dma_start(out=dden, in_=den[:, :])
            nc.sync.dma_start(out=drec, in_=rec.bitcast(FP32))
        ab01 = abp.tile([128, 512], FP32R, tag="ab01")
        ab2 = abp.tile([64, 512], FP32R, tag="ab2")
        ab_of_head = [(ab01, 0), (ab01, 64), (ab2, 0)]
        for h in range(HPC):
            rep = ps_rp.tile([128, 512], FP32, tag="rp")
            nc.tensor.matmul(
                rep[0:64, :w],
                lhsT=ones3[32 * h:32 * h + 1, 0:64],
                rhs=rec[32 * h:32 * h + 1, :w],
                start=True,
                stop=True,
            )
            abt, ab_base = ab_of_head[h]
            nc.vector.tensor_tensor(
                out=abt[ab_base:ab_base + 64, :w],
                in0=un_tiles[h][:, :w],
                in1=rep[0:64, :w],
                op=OP.mult,
            )
        if phases == 2:
            nc.sync.dma_start(out=dab[0:128, :], in_=ab01.bitcast(FP32))
            nc.sync.dma_start(out=dab[128:192, :], in_=ab2.bitcast(FP32))
            return
        # output projection for this q-tile; DMA straight from PSUM
        t0 = 0
        while t0 < w:
            tw = min(128, w - t0)
            for half in range(2):
                op_ps = ps_rp.tile([128, 512], FP32, tag="rp")
                nc.tensor.matmul(
                    op_ps[:tw, :384],
                    lhsT=ab01[:, t0:t0 + tw],
                    rhs=wo01[:, half * 384:half * 384 + 384],
                    start=True,
                    stop=False,
                )
                nc.tensor.matmul(
                    op_ps[:tw, :384],
                    lhsT=ab2[0:64, t0:t0 + tw],
                    rhs=wo2[:, half * 384:half * 384 + 384],
                    start=False,
                    stop=True,
                )
                osb = outp_pool.tile([128, 384], FP32, tag="osb")
                nc.vector.tensor_copy(osb[:tw, :], op_ps[:tw, :384])
                nc.sync.dma_start(
                    out=out[off + t0:off + t0 + tw,
                            half * 384:half * 384 + 384],
                    in_=osb[:tw, :],
                )
            t0 += tw


_NC_CACHE = {}


def build_nc(trace_sim=False, phases=3):
    key = (bool(trace_sim), phases)
    if key in _NC_CACHE:
        return _NC_CACHE[key]
    from contextlib import ExitStack

    nc = bass.Bass("TRN2", target_bir_lowering=False, debug=False, num_devices=8)
    with tile.TileContext(nc, trace_sim=trace_sim) as tc:
        with ExitStack() as ctx:
            _emit(nc, tc, ctx, phases=phases)
    split_excess_waits(nc)
    _NC_CACHE[key] = nc
    return nc


def host_prep(x, pos0, pos1, wq, wk, wv, wo, core):
    """Per-core DRAM inputs. core -> batch b=core//4, heads 3*(core%4)+[0..2]."""
    b = core // 4
    h0 = 3 * (core % 4)
    hs = [h0, h0 + 1, h0 + 2]

    xT = np.zeros((D, NP), np.float32)
    xT[:, :N] = x[b].T

    def perm_rows(w_h):  # evens then odds of the head dim
        return np.concatenate([w_h[0::2], w_h[1::2]], axis=0)

    wq_rows = np.concatenate([perm_rows(wq[h * E:(h + 1) * E]) for h in hs], 0)
    wk_rows = np.concatenate([perm_rows(wk[h * E:(h + 1) * E]) for h in hs], 0)
    wv_rows = np.concatenate([wv[h * E:(h + 1) * E] for h in hs], 0)
    wqkvT = np.ascontiguousarray(np.concatenate([wq_rows, wk_rows, wv_rows], 0).T)

    wo_cols = np.concatenate([wo[:, h * E:(h + 1) * E] for h in hs], 1)
    woT = np.ascontiguousarray(wo_cols.T)

    theta = 1.0 / (BASE ** (np.arange(KMAX, dtype=np.float32) / KMAX))
    i1, i2 = np.meshgrid(np.arange(N1), np.arange(N2), indexing="ij")
    ang0 = pos0[b][i1.ravel()][:, None] * theta[None, :]
    ang1 = pos1[b][i2.ravel()][:, None] * theta[None, :]
    ang = np.concatenate([ang0, ang1], 1).astype(np.float32)  # [N-1, 32]
    cos = np.ones((32, NP), np.float32)   # col 0 (CLS) and pad col: identity
    sin = np.zeros((32, NP), np.float32)
    cos[:, 1:N] = np.cos(ang).T
    sin[:, 1:N] = np.sin(ang).T
    cc = np.ascontiguousarray(np.tile(cos, (4, 1)))  # [128, NP]
    ss = np.ascontiguousarray(np.tile(sin, (4, 1)))
    onesd = np.ones((65, 64), np.float32)
    onescol = np.ones((128, 19, HPC), np.float32)
    onescol[1:, 18, :] = 0.0  # pad key contributes nothing
    return {"xT": xT, "wqkvT": wqkvT, "woT": woT, "cc": cc, "ss": ss,
            "onesd": onesd, "onescol": np.ascontiguousarray(onescol.reshape(128, 57))}


def kernel(x, pos0, pos1, wq, wk, wv, wo):
    x = np.asarray(x, np.float32)
    pos0 = np.asarray(pos0, np.float32)
    pos1 = np.asarray(pos1, np.float32)
    wq = np.asarray(wq, np.float32)
    wk = np.asarray(wk, np.float32)
    wv = np.asarray(wv, np.float32)
    wo = np.asarray(wo, np.float32)

    in_maps = [
        host_prep(x, pos0, pos1, wq, wk, wv, wo, core) for core in range(8)
    ]
    nc = build_nc()
    res = run_bass_kernel_spmd(nc, in_maps, core_ids=list(range(8)))
    out = np.zeros((B, N, D), np.float32)
    for core in range(8):
        out[core // 4] += res.results[core]["out"][:N]
    return out
